# revision 33
# baseline (speedup 1.0000x reference)
"""Transformer-XL block (relative-position attention + MLP) on 8 TRN2 NeuronCores.

Sharding: core c handles batch b = c//2, query rows [lo, lo+256), lo = 256*(c%2).
Each core independently computes its 256 output rows (data-parallel over (b, q-half));
k/v/r projections are recomputed per core (no collectives needed).

Math per core (all matmuls bf16 operands, fp32 PSUM accumulation):
  qT[hk, 257]= Wq^T xq^T          (257 = 256 local rows + 1 halo row for rel_shift)
  rT[hk, kl] = Wr^T pe^T
  kT[hk, kl] = Wk^T kv^T          (lhsT=Wk[dchunk, hk], rhs=kvT[dchunk, kl])
  v[kl, hk]  = kv Wv              (lhsT=kvT[dchunk, klchunk], rhs=Wv[dchunk, hk])
  halo BD rows (one per head, q row lo+256) precomputed before the head loop
  per head h:
    qcT = (qT_h + cb_h) * 0.125 ; qpT = (qT_h + pb_h) * 0.125
    BDraw     = qpT^T rT_h  -> write padded rows to DRAM y_h ([257,1025], col0=0)
    BDshift   = contiguous read of y.flat[sb + 1024*row : ...]  (sb = 512-lo, per-core
                dynamic register offset; rel_shift == overlapping strided view)
    AC[q,kl]  = qcT^T kT_h  (psum, evicted immediately to bf16)
    S = AC + BDshift (+ mask) ; P = exp(S) (no max-sub; scores are O(1)) ; rowsum
    Pn = P / rowsum ; PT = transpose(Pn) ; ctxT_h[64, 256] = sum_kc v_h^T PTchunks
  out1[q, D] = sum ctxT^T Wo ; u = x + out1 ; y = LN1(u)
  h1T[f,q] = relu(W1^T yT + b1) ; out2[q,D] = sum h1T^T W2 ; u2 = y + out2 (+b2)
  out = LN2(u2)  (identity gamma/beta and all-ones non-pad-mask are compiled out)

PSUM discipline: every attention psum tile is <= 1 bank; pools sized so all four
attention pools fit the 8 banks, letting 2+ heads stay in flight (keeps the PE's
HAM clock-gate warm -- idle gaps >3.4us would halve the PE clock).
"""

import numpy as np

import concourse.bass as bass
import concourse.tile as tile
from concourse.tile import add_dep_helper
from concourse import bacc, mybir
from concourse.bass_utils import run_bass_kernel_spmd
from concourse.masks import make_identity

F32 = mybir.dt.float32
BF16 = mybir.dt.bfloat16
U32 = mybir.dt.uint32
NP_BF16 = mybir.dt.np(BF16)

B, Q, M, D, H, DH = 4, 512, 512, 1024, 16, 64
KL = M + Q            # 1024
QL = 256              # local q rows per core
HK = H * DH           # 1024
F = 4 * D             # 4096
P = 128
NCORES = 8
YW = KL + 1           # 1025, padded y row width
YROWS = QL + 1        # 257
EPS = 1e-5

_cache = {}


def mm_acc(nc, psum, lhsT, rhs, first, last, nmax=512):
    """matmul psum += lhsT.T @ rhs, splitting the moving free dim to <=512
    (one PSUM bank per matmul instruction)."""
    n = rhs.shape[-1]
    for o in range(0, n, nmax):
        w = min(nmax, n - o)
        nc.tensor.matmul(
            psum[:, o : o + w], lhsT, rhs[:, o : o + w], start=first, stop=last
        )


def build(flags):
    """flags: (use_mask, use_npm, use_g1, use_be1, use_g2, use_be2, use_b2)"""
    use_mask, use_npm, use_g1, use_be1, use_g2, use_be2, use_b2 = flags
    nc = bacc.Bacc(None, target_bir_lowering=False)

    # ---------------- I/O ----------------
    sb_t = nc.dram_tensor("sb", [1, 1], U32, kind="ExternalInput")
    xqT = nc.dram_tensor("xqT", [D, YROWS], BF16, kind="ExternalInput")
    kvT = nc.dram_tensor("kvT", [D, KL], BF16, kind="ExternalInput")
    peT = nc.dram_tensor("peT", [D, KL], BF16, kind="ExternalInput")
    xres = nc.dram_tensor("xres", [QL, D], F32, kind="ExternalInput")
    Wq = nc.dram_tensor("Wq", [D, HK], BF16, kind="ExternalInput")
    Wk = nc.dram_tensor("Wk", [D, HK], BF16, kind="ExternalInput")
    Wv = nc.dram_tensor("Wv", [D, HK], BF16, kind="ExternalInput")
    Wr = nc.dram_tensor("Wr", [D, HK], BF16, kind="ExternalInput")
    Wo = nc.dram_tensor("Wo", [HK, D], BF16, kind="ExternalInput")
    W1 = nc.dram_tensor("W1", [D, F], BF16, kind="ExternalInput")
    W2 = nc.dram_tensor("W2", [F, D], BF16, kind="ExternalInput")
    cbt = nc.dram_tensor("cbt", [P, HK // P], F32, kind="ExternalInput")
    pbt = nc.dram_tensor("pbt", [P, HK // P], F32, kind="ExternalInput")
    b1t = nc.dram_tensor("b1t", [P, F // P], F32, kind="ExternalInput")
    if use_mask:
        maskadd = nc.dram_tensor("maskadd", [KL, QL], F32, kind="ExternalInput")
    if use_npm:
        npmt = nc.dram_tensor("npmt", [P, QL // P], F32, kind="ExternalInput")
    row_vecs = {}
    for name, used in (
        ("g1r", use_g1), ("be1r", use_be1), ("g2r", use_g2),
        ("be2r", use_be2), ("b2r", use_b2),
    ):
        if used:
            row_vecs[name] = nc.dram_tensor(name, [1, D], F32, kind="ExternalInput")
    out_t = nc.dram_tensor("out", [QL, D], F32, kind="ExternalOutput")

    def bcast_row(t):
        # [1, D] dram -> [[0,P],[1,D]] broadcast AP over partitions
        return bass.AP(tensor=t.tensor, offset=t.offset, ap=[[0, P], [1, D]])

    with tile.TileContext(nc) as tc:
        # shift base register (512 - lo), per-core input
        regs = nc.alloc_registers("sbase")
        nc.regs_load(regs, sb_t[0:1, 0:1])
        sb_sv = nc.snap(regs, donate=True, min_val=0, max_val=512)

        with (
            tc.tile_pool(name="const", bufs=1) as const,
            tc.tile_pool(name="acts", bufs=1) as acts,
        ):
            ident = const.tile([P, P], BF16)
            make_identity(nc, ident)
            ones_m = const.tile([P, P], BF16)
            nc.vector.memset(ones_m, 1.0)
            eps_t = const.tile([P, 1], F32)
            nc.vector.memset(eps_t, EPS)
            cb_s = const.tile([P, HK // P], F32)
            pb_s = const.tile([P, HK // P], F32)
            b1_s = const.tile([P, F // P], F32)
            if use_npm:
                npm_s = const.tile([P, QL // P], F32)
            rv = {}
            for name in row_vecs:
                rv[name] = const.tile([P, D], F32, name=f"rv_{name}", tag=f"rv_{name}")
            if use_mask:
                mask_s = const.tile([P, 8, QL], F32)
            xres_s = const.tile([P, 2, D], F32)

            # ------- persistent activation tensors -------
            # kT/rT/v/qT live only through attention; their pool closes after
            # phase B so phases C/D can reuse the 52KB/partition.
            abuf_ctx = tc.tile_pool(name="abuf", bufs=1)
            abuf = abuf_ctx.__enter__()
            kT_s = abuf.tile([P, 8, KL], BF16)    # [hk-chunk rows, chunk, kl]
            rT_s = abuf.tile([P, 8, KL], BF16)
            v_s = abuf.tile([P, 8, HK], BF16)     # [kl-chunk rows, chunk, hk]
            qT_s = abuf.tile([P, 8, YROWS], BF16)
            ctxT_s = acts.tile([P, 8, QL], BF16)  # [hk-chunk rows, chunk, q]
            halo_s = acts.tile([1, H, YW], BF16)  # precomputed halo BD rows (col0=0)
            y_s = acts.tile([P, 2, D], F32)       # post-LN1
            yT_s = acts.tile([P, 8, QL], BF16)    # y transposed

            # =================== PHASE A: projections ===================
            with (
                tc.tile_pool(name="pa_w", bufs=3) as pa_w,
                tc.tile_pool(name="pa_x", bufs=1) as pa_x,
                tc.tile_pool(name="pa_ps", bufs=3, space="PSUM") as pa_ps,
            ):
                # Queue plan (FIFO per engine queue, critical-path first):
                #   sync:   xqT, peT[4:8], Wr wm   scalar: Wq wm, Wk wm, peT[0:4]
                #   gpsimd: kvT, Wv
                xqT_s = pa_x.tile([P, 8, YROWS], BF16, name="xqT_s", tag="xqT_s")
                nc.scalar.dma_start(out=xqT_s, in_=xqT.rearrange("(c p) k -> p c k", p=P))
                wq_s = pa_x.tile([P, 8, HK], BF16, name="wq_s", tag="wq_s")
                wq_r = Wq.rearrange("(c p) n -> p c n", p=P)
                nc.scalar.dma_start(out=wq_s[:, 0:2, :], in_=wq_r[:, 0:2, :])
                nc.gpsimd.dma_start(out=wq_s[:, 2:4, :], in_=wq_r[:, 2:4, :])
                nc.scalar.dma_start(out=wq_s[:, 4:6, :], in_=wq_r[:, 4:6, :])
                nc.scalar.dma_start(out=wq_s[:, 6:8, :], in_=wq_r[:, 6:8, :])
                kvT_s = pa_x.tile([P, 8, KL], BF16, name="kvT_s", tag="kvT_s")
                kvT_r = kvT.rearrange("(c p) k -> p c k", p=P)
                nc.gpsimd.dma_start(out=kvT_s[:, 0:4, :], in_=kvT_r[:, 0:4, :])
                nc.scalar.dma_start(out=kvT_s[:, 4:8, :], in_=kvT_r[:, 4:8, :])
                peT_s = pa_x.tile([P, 8, KL], BF16, name="peT_s", tag="peT_s")
                peT_r = peT.rearrange("(c p) k -> p c k", p=P)
                nc.gpsimd.dma_start(out=peT_s[:, 0:4, :], in_=peT_r[:, 0:4, :])
                nc.gpsimd.dma_start(out=peT_s[:, 4:8, :], in_=peT_r[:, 4:8, :])
                wv_s = pa_x.tile([P, 8, HK], BF16, name="wv_s", tag="wv_s")
                wv_r = Wv.rearrange("(c p) n -> p c n", p=P)
                nc.gpsimd.dma_start(out=wv_s[:, 0:4, :], in_=wv_r[:, 0:4, :])
                nc.gpsimd.dma_start(out=wv_s[:, 4:8, :], in_=wv_r[:, 4:8, :])
                # deferred small const loads (off the critical startup path)
                nc.gpsimd.dma_start(out=cb_s, in_=cbt[:])
                nc.gpsimd.dma_start(out=pb_s, in_=pbt[:])
                nc.gpsimd.dma_start(out=b1_s, in_=b1t[:])
                if use_npm:
                    nc.gpsimd.dma_start(out=npm_s, in_=npmt[:])
                for name in row_vecs:
                    nc.gpsimd.dma_start(out=rv[name], in_=bcast_row(row_vecs[name]))
                if use_mask:
                    nc.scalar.dma_start(
                        out=mask_s, in_=maskadd.rearrange("(c p) a -> p c a", p=P)
                    )
                nc.gpsimd.dma_start(
                    out=xres_s, in_=xres.rearrange("(t p) d -> p t d", p=P)
                )

                def proj(dst, w_dram, rhs_s, ncols, wname, dve_evict, dma_eng,
                         w_pre=None):
                    # dst[p, m, :ncols] (8 m-chunks of 128) = W^T @ rhs
                    for m in range(8):
                        if w_pre is None:
                            wm = pa_w.tile([P, 8, P], BF16, name=f"w_{wname}{m}", tag="wm")
                            dma_eng.dma_start(
                                out=wm,
                                in_=w_dram.rearrange("(c p) n -> p c n", p=P)[
                                    :, :, m * P : (m + 1) * P
                                ],
                            )
                        ps = pa_ps.tile([P, KL], F32, name=f"ps_{wname}{m}", tag="pa_psum")
                        for kc in range(8):
                            lhsT = (
                                w_pre[:, kc, m * P : (m + 1) * P]
                                if w_pre is not None else wm[:, kc, :]
                            )
                            mm_acc(
                                nc, ps[:, :ncols], lhsT, rhs_s[:, kc, :ncols],
                                first=(kc == 0), last=(kc == 7),
                            )
                        if dve_evict:
                            nc.vector.tensor_copy(dst[:, m, :ncols], ps[:, :ncols])
                        else:
                            nc.scalar.copy(out=dst[:, m, :ncols], in_=ps[:, :ncols])

                proj(qT_s, Wq, xqT_s, YROWS, "q", False, None, w_pre=wq_s)
                proj(kT_s, Wk, kvT_s, KL, "k", True, nc.scalar)
                proj(rT_s, Wr, peT_s, KL, "r", True, nc.gpsimd)
                # v = kv @ Wv : lhsT = kvT chunk [128d, 128kl], rhs = Wv [128d, hk]
                for m in range(8):  # kl-chunks
                    ps = pa_ps.tile([P, HK], F32, name=f"ps_v{m}", tag="pa_psum")
                    for kc in range(8):
                        mm_acc(
                            nc, ps, kvT_s[:, kc, m * P : (m + 1) * P],
                            wv_s[:, kc, :],
                            first=(kc == 0), last=(kc == 7),
                        )
                    nc.vector.tensor_copy(v_s[:, m, :], ps)

            # ====== PHASE A2: halo BD rows (q row lo+256, one per head) ======
            with (
                tc.tile_pool(name="ph_w", bufs=2) as ph_w,
                tc.tile_pool(name="ph_ps", bufs=2, space="PSUM") as ph_ps,
            ):
                qph = ph_w.tile([P, 8], BF16, name="qph", tag="qph")
                nc.vector.memset(halo_s[0:1, :, 0:1], 0.0)
                for hi in range(8):
                    nc.vector.tensor_scalar(
                        out=qph[:, hi : hi + 1], in0=qT_s[:, hi, QL : QL + 1],
                        scalar1=pb_s[:, hi : hi + 1], scalar2=0.125,
                        op0=mybir.AluOpType.add, op1=mybir.AluOpType.mult,
                    )
                for h in range(H):
                    hi, hr = h // 2, (h % 2) * DH
                    psh = ph_ps.tile([1, KL], F32, name="psh", tag="psh")
                    mm_acc(nc, psh, qph[hr : hr + DH, hi : hi + 1],
                           rT_s[hr : hr + DH, hi, :], first=True, last=True)
                    nc.scalar.copy(out=halo_s[0:1, h, 1:], in_=psh)

            # =================== PHASE B: attention ===================
            with (
                tc.tile_pool(name="pb_work", bufs=3) as work,
                tc.tile_pool(name="pb_dram", bufs=1, space="DRAM") as ydram,
                tc.tile_pool(name="pb_ps_sc", bufs=2, space="PSUM") as ps_sc,
                tc.tile_pool(name="pb_ps_sa", bufs=2, space="PSUM") as ps_sa,
                tc.tile_pool(name="pb_ps_ms", bufs=1, space="PSUM") as ps_ms,
            ):
                ad_hist = []  # per-head list of sT add instrs (bdT readers)
                # prefetch Wo during attention (used in phase C)
                wo_s = acts.tile([P, 8, D], BF16, name="wo_s", tag="wo_s")
                wo_r = Wo.rearrange("(c p) n -> p c n", p=P)
                nc.gpsimd.dma_start(out=wo_s[:, 0:4, :], in_=wo_r[:, 0:4, :])
                nc.gpsimd.dma_start(out=wo_s[:, 4:8, :], in_=wo_r[:, 4:8, :])

                for h in range(H):
                    hi, hr = h // 2, (h % 2) * DH
                    kT_h = kT_s[hr : hr + DH, hi, :]
                    rT_h = rT_s[hr : hr + DH, hi, :]
                    qT_h = qT_s[hr : hr + DH, hi, :]

                    qcT_f = work.tile([P, QL], BF16, name="qcT_f", tag="qcT_f")
                    qcT = qcT_f[hr : hr + DH, :]
                    nc.vector.tensor_scalar(
                        out=qcT, in0=qT_h[:, :QL],
                        scalar1=cb_s[hr : hr + DH, hi : hi + 1], scalar2=0.125,
                        op0=mybir.AluOpType.add, op1=mybir.AluOpType.mult,
                    )
                    qpT_f = work.tile([P, QL], BF16, name="qpT_f", tag="qpT_f")
                    qpT = qpT_f[hr : hr + DH, :]
                    nc.vector.tensor_scalar(
                        out=qpT, in0=qT_h[:, :QL],
                        scalar1=pb_s[hr : hr + DH, hi : hi + 1], scalar2=0.125,
                        op0=mybir.AluOpType.add, op1=mybir.AluOpType.mult,
                    )

                    # --- BD_raw -> y_h (row layout, padded rows, col0 = 0) ---
                    y1 = ydram.tile(
                        [(YROWS + 1) * YW], BF16, name=f"y{h}", tag=f"y{h}"
                    )
                    y2d = y1.rearrange("(a b) -> a b", b=YW)
                    wr_insts = []
                    for t in range(2):
                        bw = work.tile([P, YW], BF16, name="bw", tag="bw")
                        nc.gpsimd.memset(bw[:, 0:1], 0.0)
                        for o in range(0, KL, 512):
                            psb = ps_sc.tile([P, 512], F32, name="psb", tag="sc")
                            nc.tensor.matmul(
                                psb, qpT[:, t * P : (t + 1) * P],
                                rT_h[:, o : o + 512], start=True, stop=True,
                            )
                            nc.scalar.copy(out=bw[:, 1 + o : 513 + o], in_=psb)
                        wr_insts.append(
                            nc.gpsimd.dma_start(
                                out=y2d[t * P : (t + 1) * P, :], in_=bw
                            )
                        )
                    wr_insts.append(
                        nc.gpsimd.dma_start(
                            out=y2d[QL : QL + 1, :], in_=halo_s[0:1, h, :]
                        )
                    )

                    # --- transposed BD read via one DMA xbar transpose:
                    # in [256a, 1024kb] -> out [128p, 8c, 256a] (3D out folds the
                    # extra dim into logical partitions). dma_start_transpose
                    # lowers its APs eagerly, so Tile cannot track the y1/bdT
                    # accesses -- add the write->read->use deps explicitly.
                    bdT = work.tile([P, 8, QL], BF16, name="bdT", tag="bdT")
                    tr_inst = nc.sync.dma_start(
                        out=bdT,
                        in_=y1[bass.ds(sb_sv, QL * KL)].rearrange(
                            "(a b) -> a b", b=KL
                        ),
                        transpose=True,
                    )
                    for wi in wr_insts:
                        add_dep_helper(tr_inst.ins, wi.ins, reason="bdT read waits y writes")
                    if len(ad_hist) >= 3:
                        # bdT tiles rotate through 3 slots; the untracked
                        # transpose write must wait for slot h-3's readers
                        for prev_ad in ad_hist[-3]:
                            add_dep_helper(
                                tr_inst.ins, prev_ad.ins,
                                reason="bdT slot reuse waits prior readers",
                            )
                    cur_ads = []
                    ad_hist.append(cur_ads)

                    # --- S^T = AC^T + BDshift^T, exp, sums via ones-matmul ---
                    pT = work.tile([P, 8, QL], BF16, name="pT", tag="pT")
                    sT = work.tile([P, 8, QL], BF16, name="sT", tag="sT")
                    for cb4 in range(2):
                        psa = ps_sa.tile([P, 4, QL], F32, name="psa", tag="sa")
                        for cc in range(4):
                            c = cb4 * 4 + cc
                            nc.tensor.matmul(
                                psa[:, cc, :], kT_h[:, c * P : (c + 1) * P], qcT,
                                start=True, stop=True,
                            )
                        ad = nc.vector.tensor_add(
                            out=sT[:, cb4 * 4 : cb4 * 4 + 4, :], in0=psa,
                            in1=bdT[:, cb4 * 4 : cb4 * 4 + 4, :],
                        )
                        add_dep_helper(ad.ins, tr_inst.ins, reason="s add waits bdT read")
                        cur_ads.append(ad)
                        if use_mask:
                            nc.vector.tensor_add(
                                out=sT[:, cb4 * 4 : cb4 * 4 + 4, :],
                                in0=sT[:, cb4 * 4 : cb4 * 4 + 4, :],
                                in1=mask_s[:, cb4 * 4 : cb4 * 4 + 4, :],
                            )
                    nc.scalar.activation(
                        out=pT, in_=sT, func=mybir.ActivationFunctionType.Exp
                    )
                    pssum = ps_ms.tile([P, QL], F32, name="pssum", tag="msum")
                    for c in range(8):
                        nc.tensor.matmul(
                            pssum, ones_m, pT[:, c, :],
                            start=(c == 0), stop=(c == 7),
                        )
                    rcpb = work.tile([P, QL], F32, name="rcpb", tag="rcpb")
                    nc.vector.reciprocal(out=rcpb, in_=pssum)

                    # --- ctxT_h = (v_h^T @ pT) * rcpb ---
                    psc_f = ps_ms.tile([P, QL], F32, name="psc_f", tag="cx")
                    psc = psc_f[hr : hr + DH, :]
                    for j in range(8):
                        nc.tensor.matmul(
                            psc, v_s[:, j, h * DH : (h + 1) * DH], pT[:, j, :],
                            start=(j == 0), stop=(j == 7),
                        )
                    nc.vector.tensor_mul(
                        out=ctxT_s[hr : hr + DH, hi, :], in0=psc,
                        in1=rcpb[hr : hr + DH, :],
                    )

            abuf_ctx.__exit__(None, None, None)

            # =================== PHASE C: Wo + LN1 ===================
            with (
                tc.tile_pool(name="pc_w", bufs=3) as pc_w,
                tc.tile_pool(name="pc_work", bufs=2) as cwork,
                tc.tile_pool(name="pc_ps", bufs=2, space="PSUM") as pc_ps,
                tc.tile_pool(name="pc_ps_tp", bufs=2, space="PSUM") as pc_ps_tp,
            ):

                def layer_norm(dst, u, gname, bname):
                    # dst, u: [P, D] f32 sbuf aps
                    stats = cwork.tile([P, 2, 6], F32, name="stats", tag="stats")
                    for sg in range(2):
                        nc.vector.bn_stats(
                            out=stats[:, sg, :], in_=u[:, sg * 512 : (sg + 1) * 512]
                        )
                    mv = cwork.tile([P, 2], F32, name="mv", tag="mv")
                    nc.vector.bn_aggr(out=mv, in_=stats)
                    rstd = cwork.tile([P, 1], F32, name="rstd", tag="rstd")
                    nc.scalar.activation(
                        out=rstd, in_=mv[:, 1:2],
                        func=mybir.ActivationFunctionType.Sqrt, bias=eps_t,
                    )
                    nc.vector.reciprocal(out=rstd, in_=rstd)
                    nc.vector.tensor_scalar(
                        out=dst, in0=u, scalar1=mv[:, 0:1], scalar2=rstd,
                        op0=mybir.AluOpType.subtract, op1=mybir.AluOpType.mult,
                    )
                    if gname:
                        nc.vector.tensor_mul(out=dst, in0=dst, in1=rv[gname])
                    if bname:
                        nc.vector.tensor_add(out=dst, in0=dst, in1=rv[bname])

                for t in range(2):
                    pso = pc_ps.tile([P, D], F32, name="pso", tag="pso")
                    for j in range(8):
                        mm_acc(nc, pso, ctxT_s[:, j, t * P : (t + 1) * P],
                               wo_s[:, j, :], first=(j == 0), last=(j == 7))
                    u1 = cwork.tile([P, D], F32, name="u1", tag="u1")
                    nc.vector.tensor_add(out=u1, in0=pso, in1=xres_s[:, t, :])
                    layer_norm(
                        y_s[:, t, :], u1,
                        "g1r" if use_g1 else None, "be1r" if use_be1 else None,
                    )
                    if use_npm:
                        nc.vector.tensor_scalar_mul(
                            out=y_s[:, t, :], in0=y_s[:, t, :],
                            scalar1=npm_s[:, t : t + 1],
                        )
                    ybf = cwork.tile([P, D], BF16, name="ybf", tag="ybf")
                    nc.scalar.copy(out=ybf, in_=y_s[:, t, :])
                    for j in range(8):
                        tp = pc_ps_tp.tile([P, P], BF16, name="tp2", tag="tp2")
                        nc.tensor.transpose(tp, ybf[:, j * P : (j + 1) * P], ident)
                        nc.scalar.copy(out=yT_s[:, j, t * P : (t + 1) * P], in_=tp)

                # =================== PHASE D: MLP + LN2 ===================
                with (
                    tc.tile_pool(name="pd_w", bufs=3) as pd_w,
                    tc.tile_pool(name="pd_h", bufs=1) as pd_h,
                    tc.tile_pool(name="pd_ps_h", bufs=2, space="PSUM") as pd_ps_h,
                ):
                    h1T = pd_h.tile([P, F // P, QL], BF16)
                    for jb in range(8):  # batches of 4 f-chunks
                        w1 = pd_w.tile([P, 8, 512], BF16, name="w1", tag="w1")
                        (nc.scalar, nc.gpsimd)[jb % 2].dma_start(
                            out=w1,
                            in_=W1.rearrange("(c p) n -> p c n", p=P)[
                                :, :, jb * 512 : (jb + 1) * 512
                            ],
                        )
                        for jj in range(4):
                            j = jb * 4 + jj
                            psh1 = pd_ps_h.tile([P, QL], F32, name="psh1", tag="psh1")
                            for kc in range(8):
                                nc.tensor.matmul(
                                    psh1, w1[:, kc, jj * P : (jj + 1) * P],
                                    yT_s[:, kc, :],
                                    start=(kc == 0), stop=(kc == 7),
                                )
                            nc.scalar.activation(
                                out=h1T[:, j, :], in_=psh1,
                                func=mybir.ActivationFunctionType.Relu,
                                bias=b1_s[:, j : j + 1],
                            )
                    ps2_t = [
                        pc_ps.tile([P, D], F32, name=f"ps2_{t}", tag="pso")
                        for t in range(2)
                    ]
                    for jb in range(8):
                        w2 = pd_w.tile([P, 4, D], BF16, name="w2", tag="w2")
                        (nc.gpsimd, nc.scalar)[jb % 2].dma_start(
                            out=w2,
                            in_=W2.rearrange("(c p) n -> p c n", p=P)[
                                :, jb * 4 : (jb + 1) * 4, :
                            ],
                        )
                        for jj in range(4):
                            j = jb * 4 + jj
                            for t in range(2):
                                mm_acc(nc, ps2_t[t], h1T[:, j, t * P : (t + 1) * P],
                                       w2[:, jj, :],
                                       first=(j == 0), last=(j == F // P - 1))
                    for t in range(2):
                        ps2 = ps2_t[t]
                        u2 = cwork.tile([P, D], F32, name="u2", tag="u1")
                        nc.vector.tensor_add(out=u2, in0=ps2, in1=y_s[:, t, :])
                        if use_b2:
                            nc.vector.tensor_add(out=u2, in0=u2, in1=rv["b2r"])
                        o2 = cwork.tile([P, D], F32, name="o2", tag="o2")
                        layer_norm(
                            o2, u2,
                            "g2r" if use_g2 else None, "be2r" if use_be2 else None,
                        )
                        if use_npm:
                            nc.vector.tensor_scalar_mul(
                                out=o2, in0=o2, scalar1=npm_s[:, t : t + 1]
                            )
                        nc.gpsimd.dma_start(
                            out=out_t.rearrange("(t p) d -> p t d", p=P)[:, t, :],
                            in_=o2,
                        )

    nc.compile()
    return nc


def _host_prep(inputs):
    """Shared (core-independent) host prep: bf16 casts + pe table."""
    f32 = lambda x: np.asarray(x, np.float32)
    bf = lambda x: np.asarray(x, np.float32).astype(NP_BF16)

    pos = np.arange(KL - 1, -1, -1, dtype=np.float32)
    inv = (1.0 / (10000.0 ** (np.arange(0, D, 2, dtype=np.float32) / D))).astype(
        np.float32
    )
    ang = pos[:, None] * inv
    pe = np.concatenate([np.sin(ang), np.cos(ang)], axis=-1).astype(np.float32)

    shared = {
        "peT": bf(pe.T),
        "Wq": bf(inputs["Wq"]), "Wk": bf(inputs["Wk"]),
        "Wv": bf(inputs["Wv"]), "Wr": bf(inputs["Wr"]),
        "Wo": bf(inputs["Wo"]), "W1": bf(inputs["W1"]), "W2": bf(inputs["W2"]),
        "cbt": f32(inputs["content_bias"]).reshape(HK).reshape(8, P).T.copy(),
        "pbt": f32(inputs["position_bias"]).reshape(HK).reshape(8, P).T.copy(),
        "b1t": f32(inputs["b1"]).reshape(F // P, P).T.copy(),
    }
    return shared


def _run(inputs, trace=False):
    x = np.asarray(inputs["layer_input"], np.float32)
    mem = np.asarray(inputs["memory"], np.float32)
    npm = np.asarray(inputs["non_pad_mask"], np.float32)
    mask = np.asarray(inputs["slf_attn_mask"])
    g1 = np.asarray(inputs["ln1_g"], np.float32)
    be1 = np.asarray(inputs["ln1_b"], np.float32)
    g2 = np.asarray(inputs["ln2_g"], np.float32)
    be2 = np.asarray(inputs["ln2_b"], np.float32)
    b2 = np.asarray(inputs["b2"], np.float32)

    flags = (
        bool(mask.any()),
        not bool(np.all(npm == 1.0)),
        not bool(np.all(g1 == 1.0)),
        bool(be1.any()),
        not bool(np.all(g2 == 1.0)),
        bool(be2.any()),
        bool(b2.any()),
    )
    if flags not in _cache:
        _cache[flags] = build(flags)
    nc = _cache[flags]
    use_mask, use_npm, use_g1, use_be1, use_g2, use_be2, use_b2 = flags

    shared = _host_prep(inputs)
    in_maps = []
    for c in range(NCORES):
        b, qh = c // 2, c % 2
        lo = QL * qh
        kv = np.concatenate([mem[b], x[b]], axis=0)  # [KL, D]
        xq = np.zeros((YROWS, D), np.float32)
        hi = min(lo + YROWS, Q)
        xq[: hi - lo] = x[b, lo:hi]
        m = dict(shared)
        m["sb"] = np.array([[512 - lo]], np.uint32)
        m["xqT"] = xq.T.astype(NP_BF16)
        m["kvT"] = kv.T.astype(NP_BF16)
        m["xres"] = np.ascontiguousarray(x[b, lo : lo + QL])
        if use_mask:
            m["maskadd"] = np.where(
                mask[b, lo : lo + QL], np.float32(-1e9), np.float32(0)
            ).astype(np.float32)
        if use_npm:
            m["npmt"] = npm[b, lo : lo + QL, 0].reshape(2, P).T.copy()
        if use_g1:
            m["g1r"] = g1.reshape(1, D).copy()
        if use_be1:
            m["be1r"] = be1.reshape(1, D).copy()
        if use_g2:
            m["g2r"] = g2.reshape(1, D).copy()
        if use_be2:
            m["be2r"] = be2.reshape(1, D).copy()
        if use_b2:
            m["b2r"] = b2.reshape(1, D).copy()
        in_maps.append(m)

    res = run_bass_kernel_spmd(nc, in_maps, core_ids=list(range(NCORES)), trace=trace)
    out = np.empty((B, Q, D), np.float32)
    for c in range(NCORES):
        b, qh = c // 2, c % 2
        out[b, QL * qh : QL * (qh + 1)] = res.results[c]["out"]
    return out, res


def kernel(**inputs):
    out, _ = _run(inputs, trace=False)
    return out


# revision 35
# speedup vs baseline: 1.0589x; 1.0589x over previous
"""Transformer-XL block (relative-position attention + MLP) on 8 TRN2 NeuronCores.

Sharding: core c handles batch b = c//2, query rows [lo, lo+256), lo = 256*(c%2).
Each core independently computes its 256 output rows (data-parallel over (b, q-half));
k/v/r projections are recomputed per core (no collectives needed).

Math per core (all matmuls bf16 operands, fp32 PSUM accumulation):
  qT[hk, 257]= Wq^T xq^T          (257 = 256 local rows + 1 halo row for rel_shift)
  rT[hk, kl] = Wr^T pe^T
  kT[hk, kl] = Wk^T kv^T          (lhsT=Wk[dchunk, hk], rhs=kvT[dchunk, kl])
  v[kl, hk]  = kv Wv              (lhsT=kvT[dchunk, klchunk], rhs=Wv[dchunk, hk])
  halo BD rows (one per head, q row lo+256) precomputed before the head loop
  per head h:
    qcT = (qT_h + cb_h) * 0.125 ; qpT = (qT_h + pb_h) * 0.125
    BDraw     = qpT^T rT_h  -> write padded rows to DRAM y_h ([257,1025], col0=0)
    BDshift   = contiguous read of y.flat[sb + 1024*row : ...]  (sb = 512-lo, per-core
                dynamic register offset; rel_shift == overlapping strided view)
    AC[q,kl]  = qcT^T kT_h  (psum, evicted immediately to bf16)
    S = AC + BDshift (+ mask) ; P = exp(S) (no max-sub; scores are O(1)) ; rowsum
    Pn = P / rowsum ; PT = transpose(Pn) ; ctxT_h[64, 256] = sum_kc v_h^T PTchunks
  out1[q, D] = sum ctxT^T Wo ; u = x + out1 ; y = LN1(u)
  h1T[f,q] = relu(W1^T yT + b1) ; out2[q,D] = sum h1T^T W2 ; u2 = y + out2 (+b2)
  out = LN2(u2)  (identity gamma/beta and all-ones non-pad-mask are compiled out)

PSUM discipline: every attention psum tile is <= 1 bank; pools sized so all four
attention pools fit the 8 banks, letting 2+ heads stay in flight (keeps the PE's
HAM clock-gate warm -- idle gaps >3.4us would halve the PE clock).
"""

import numpy as np

import concourse.bass as bass
import concourse.tile as tile
from concourse.tile import add_dep_helper
from concourse import bacc, mybir
from concourse.bass_utils import run_bass_kernel_spmd
from concourse.masks import make_identity

F32 = mybir.dt.float32
BF16 = mybir.dt.bfloat16
U32 = mybir.dt.uint32
NP_BF16 = mybir.dt.np(BF16)

B, Q, M, D, H, DH = 4, 512, 512, 1024, 16, 64
KL = M + Q            # 1024
QL = 256              # local q rows per core
HK = H * DH           # 1024
F = 4 * D             # 4096
P = 128
NCORES = 8
YW = KL + 1           # 1025, padded y row width
YROWS = QL + 1        # 257
EPS = 1e-5

_cache = {}


def mm_acc(nc, psum, lhsT, rhs, first, last, nmax=512):
    """matmul psum += lhsT.T @ rhs, splitting the moving free dim to <=512
    (one PSUM bank per matmul instruction)."""
    n = rhs.shape[-1]
    for o in range(0, n, nmax):
        w = min(nmax, n - o)
        nc.tensor.matmul(
            psum[:, o : o + w], lhsT, rhs[:, o : o + w], start=first, stop=last
        )


def build(flags):
    """flags: (use_mask, use_npm, use_g1, use_be1, use_g2, use_be2, use_b2)"""
    use_mask, use_npm, use_g1, use_be1, use_g2, use_be2, use_b2 = flags
    nc = bacc.Bacc(None, target_bir_lowering=False)

    # ---------------- I/O ----------------
    sb_t = nc.dram_tensor("sb", [1, 1], U32, kind="ExternalInput")
    xqT = nc.dram_tensor("xqT", [D, YROWS], BF16, kind="ExternalInput")
    kvT = nc.dram_tensor("kvT", [D, KL], BF16, kind="ExternalInput")
    peT = nc.dram_tensor("peT", [D, KL], BF16, kind="ExternalInput")
    xres = nc.dram_tensor("xres", [QL, D], F32, kind="ExternalInput")
    Wq = nc.dram_tensor("Wq", [D, HK], BF16, kind="ExternalInput")
    Wk = nc.dram_tensor("Wk", [D, HK], BF16, kind="ExternalInput")
    Wv = nc.dram_tensor("Wv", [D, HK], BF16, kind="ExternalInput")
    Wr = nc.dram_tensor("Wr", [D, HK], BF16, kind="ExternalInput")
    Wo = nc.dram_tensor("Wo", [HK, D], BF16, kind="ExternalInput")
    W1 = nc.dram_tensor("W1", [D, F], BF16, kind="ExternalInput")
    W2 = nc.dram_tensor("W2", [F, D], BF16, kind="ExternalInput")
    cbt = nc.dram_tensor("cbt", [P, HK // P], F32, kind="ExternalInput")
    pbt = nc.dram_tensor("pbt", [P, HK // P], F32, kind="ExternalInput")
    b1t = nc.dram_tensor("b1t", [P, F // P], F32, kind="ExternalInput")
    if use_mask:
        maskadd = nc.dram_tensor("maskadd", [KL, QL], F32, kind="ExternalInput")
    if use_npm:
        npmt = nc.dram_tensor("npmt", [P, QL // P], F32, kind="ExternalInput")
    row_vecs = {}
    for name, used in (
        ("g1r", use_g1), ("be1r", use_be1), ("g2r", use_g2),
        ("be2r", use_be2), ("b2r", use_b2),
    ):
        if used:
            row_vecs[name] = nc.dram_tensor(name, [1, D], F32, kind="ExternalInput")
    out_t = nc.dram_tensor("out", [QL, D], F32, kind="ExternalOutput")

    def bcast_row(t):
        # [1, D] dram -> [[0,P],[1,D]] broadcast AP over partitions
        return bass.AP(tensor=t.tensor, offset=t.offset, ap=[[0, P], [1, D]])

    with tile.TileContext(nc) as tc:
        # shift base register (512 - lo), per-core input
        regs = nc.alloc_registers("sbase")
        nc.regs_load(regs, sb_t[0:1, 0:1])
        sb_sv = nc.snap(regs, donate=True, min_val=0, max_val=512)

        with (
            tc.tile_pool(name="const", bufs=1) as const,
            tc.tile_pool(name="acts", bufs=1) as acts,
        ):
            ident = const.tile([P, P], BF16)
            make_identity(nc, ident)
            ones_m = const.tile([P, P], BF16)
            nc.vector.memset(ones_m, 1.0)
            eps_t = const.tile([P, 1], F32)
            nc.vector.memset(eps_t, EPS)
            cb_s = const.tile([P, HK // P], F32)
            pb_s = const.tile([P, HK // P], F32)
            b1_s = const.tile([P, F // P], F32)
            if use_npm:
                npm_s = const.tile([P, QL // P], F32)
            rv = {}
            for name in row_vecs:
                rv[name] = const.tile([P, D], F32, name=f"rv_{name}", tag=f"rv_{name}")
            if use_mask:
                mask_s = const.tile([P, 8, QL], F32)
            xres_s = const.tile([P, 2, D], F32)

            # ------- persistent activation tensors -------
            # kT/rT/v/qT live only through attention; their pool closes after
            # phase B so phases C/D can reuse the 52KB/partition.
            abuf_ctx = tc.tile_pool(name="abuf", bufs=1)
            abuf = abuf_ctx.__enter__()
            kT_s = abuf.tile([P, 8, KL], BF16)    # [hk-chunk rows, chunk, kl]
            rT_s = abuf.tile([P, 8, KL], BF16)
            v_s = abuf.tile([P, 8, HK], BF16)     # [kl-chunk rows, chunk, hk]
            qT_s = abuf.tile([P, 8, YROWS], BF16)
            ctxT_s = acts.tile([P, 8, QL], BF16)  # [hk-chunk rows, chunk, q]
            halo_s = acts.tile([1, H, YW], BF16)  # precomputed halo BD rows (col0=0)
            y_s = acts.tile([P, 2, D], F32)       # post-LN1
            yT_s = acts.tile([P, 8, QL], BF16)    # y transposed

            # =================== PHASE A: projections ===================
            with (
                tc.tile_pool(name="pa_w", bufs=3) as pa_w,
                tc.tile_pool(name="pa_x", bufs=1) as pa_x,
                tc.tile_pool(name="pa_ps", bufs=3, space="PSUM") as pa_ps,
            ):
                # Queue plan (FIFO per engine queue, critical-path first):
                #   sync:   xqT, peT[4:8], Wr wm   scalar: Wq wm, Wk wm, peT[0:4]
                #   gpsimd: kvT, Wv
                xqT_s = pa_x.tile([P, 8, YROWS], BF16, name="xqT_s", tag="xqT_s")
                nc.scalar.dma_start(out=xqT_s, in_=xqT.rearrange("(c p) k -> p c k", p=P))
                wq_s = pa_x.tile([P, 8, HK], BF16, name="wq_s", tag="wq_s")
                wq_r = Wq.rearrange("(c p) n -> p c n", p=P)
                nc.scalar.dma_start(out=wq_s[:, 0:2, :], in_=wq_r[:, 0:2, :])
                nc.gpsimd.dma_start(out=wq_s[:, 2:4, :], in_=wq_r[:, 2:4, :])
                nc.scalar.dma_start(out=wq_s[:, 4:6, :], in_=wq_r[:, 4:6, :])
                nc.scalar.dma_start(out=wq_s[:, 6:8, :], in_=wq_r[:, 6:8, :])
                kvT_s = pa_x.tile([P, 8, KL], BF16, name="kvT_s", tag="kvT_s")
                kvT_r = kvT.rearrange("(c p) k -> p c k", p=P)
                nc.gpsimd.dma_start(out=kvT_s[:, 0:4, :], in_=kvT_r[:, 0:4, :])
                nc.scalar.dma_start(out=kvT_s[:, 4:8, :], in_=kvT_r[:, 4:8, :])
                peT_s = pa_x.tile([P, 8, KL], BF16, name="peT_s", tag="peT_s")
                peT_r = peT.rearrange("(c p) k -> p c k", p=P)
                nc.gpsimd.dma_start(out=peT_s[:, 0:4, :], in_=peT_r[:, 0:4, :])
                nc.gpsimd.dma_start(out=peT_s[:, 4:8, :], in_=peT_r[:, 4:8, :])
                wv_s = pa_x.tile([P, 8, HK], BF16, name="wv_s", tag="wv_s")
                wv_r = Wv.rearrange("(c p) n -> p c n", p=P)
                nc.gpsimd.dma_start(out=wv_s[:, 0:4, :], in_=wv_r[:, 0:4, :])
                nc.gpsimd.dma_start(out=wv_s[:, 4:8, :], in_=wv_r[:, 4:8, :])
                # deferred small const loads (off the critical startup path)
                nc.gpsimd.dma_start(out=cb_s, in_=cbt[:])
                nc.gpsimd.dma_start(out=pb_s, in_=pbt[:])
                nc.gpsimd.dma_start(out=b1_s, in_=b1t[:])
                if use_npm:
                    nc.gpsimd.dma_start(out=npm_s, in_=npmt[:])
                for name in row_vecs:
                    nc.gpsimd.dma_start(out=rv[name], in_=bcast_row(row_vecs[name]))
                if use_mask:
                    nc.scalar.dma_start(
                        out=mask_s, in_=maskadd.rearrange("(c p) a -> p c a", p=P)
                    )
                nc.gpsimd.dma_start(
                    out=xres_s, in_=xres.rearrange("(t p) d -> p t d", p=P)
                )

                def proj(dst, w_dram, rhs_s, ncols, wname, dve_evict, dma_eng,
                         w_pre=None):
                    # dst[p, m, :ncols] (8 m-chunks of 128) = W^T @ rhs
                    for m in range(8):
                        if w_pre is None:
                            wm = pa_w.tile([P, 8, P], BF16, name=f"w_{wname}{m}", tag="wm")
                            dma_eng.dma_start(
                                out=wm,
                                in_=w_dram.rearrange("(c p) n -> p c n", p=P)[
                                    :, :, m * P : (m + 1) * P
                                ],
                            )
                        ps = pa_ps.tile([P, KL], F32, name=f"ps_{wname}{m}", tag="pa_psum")
                        for kc in range(8):
                            lhsT = (
                                w_pre[:, kc, m * P : (m + 1) * P]
                                if w_pre is not None else wm[:, kc, :]
                            )
                            mm_acc(
                                nc, ps[:, :ncols], lhsT, rhs_s[:, kc, :ncols],
                                first=(kc == 0), last=(kc == 7),
                            )
                        if dve_evict:
                            nc.vector.tensor_copy(dst[:, m, :ncols], ps[:, :ncols])
                        else:
                            nc.scalar.copy(out=dst[:, m, :ncols], in_=ps[:, :ncols])

                proj(qT_s, Wq, xqT_s, YROWS, "q", False, None, w_pre=wq_s)
                proj(kT_s, Wk, kvT_s, KL, "k", True, nc.scalar)
                proj(rT_s, Wr, peT_s, KL, "r", True, nc.gpsimd)
                # v = kv @ Wv : lhsT = kvT chunk [128d, 128kl], rhs = Wv [128d, hk]
                for m in range(8):  # kl-chunks
                    ps = pa_ps.tile([P, HK], F32, name=f"ps_v{m}", tag="pa_psum")
                    for kc in range(8):
                        mm_acc(
                            nc, ps, kvT_s[:, kc, m * P : (m + 1) * P],
                            wv_s[:, kc, :],
                            first=(kc == 0), last=(kc == 7),
                        )
                    nc.vector.tensor_copy(v_s[:, m, :], ps)

            # ====== PHASE A2: halo BD rows (q row lo+256, one per head) ======
            with (
                tc.tile_pool(name="ph_w", bufs=2) as ph_w,
                tc.tile_pool(name="ph_ps", bufs=2, space="PSUM") as ph_ps,
            ):
                qph = ph_w.tile([P, 8], BF16, name="qph", tag="qph")
                nc.vector.memset(halo_s[0:1, :, 0:1], 0.0)
                for hi in range(8):
                    nc.vector.tensor_scalar(
                        out=qph[:, hi : hi + 1], in0=qT_s[:, hi, QL : QL + 1],
                        scalar1=pb_s[:, hi : hi + 1], scalar2=0.125,
                        op0=mybir.AluOpType.add, op1=mybir.AluOpType.mult,
                    )
                for h in range(H):
                    hi, hr = h // 2, (h % 2) * DH
                    psh = ph_ps.tile([1, KL], F32, name="psh", tag="psh")
                    mm_acc(nc, psh, qph[hr : hr + DH, hi : hi + 1],
                           rT_s[hr : hr + DH, hi, :], first=True, last=True)
                    nc.scalar.copy(out=halo_s[0:1, h, 1:], in_=psh)

            # =================== PHASE B: attention ===================
            with (
                tc.tile_pool(name="pb_work", bufs=4) as work,
                tc.tile_pool(name="pb_dram", bufs=1, space="DRAM") as ydram,
                tc.tile_pool(name="pb_ps_sc", bufs=2, space="PSUM") as ps_sc,
                tc.tile_pool(name="pb_ps_sa", bufs=2, space="PSUM") as ps_sa,
                tc.tile_pool(name="pb_ps_ms", bufs=1, space="PSUM") as ps_ms,
            ):
                ad_hist = []  # per-head list of sT add instrs (bdT readers)
                # prefetch Wo during attention (used in phase C)
                wo_s = acts.tile([P, 8, D], BF16, name="wo_s", tag="wo_s")
                wo_r = Wo.rearrange("(c p) n -> p c n", p=P)
                nc.gpsimd.dma_start(out=wo_s[:, 0:4, :], in_=wo_r[:, 0:4, :])
                nc.gpsimd.dma_start(out=wo_s[:, 4:8, :], in_=wo_r[:, 4:8, :])

                for h in range(H):
                    hi, hr = h // 2, (h % 2) * DH
                    kT_h = kT_s[hr : hr + DH, hi, :]
                    rT_h = rT_s[hr : hr + DH, hi, :]
                    qT_h = qT_s[hr : hr + DH, hi, :]

                    qcT_f = work.tile([P, QL], BF16, name="qcT_f", tag="qcT_f")
                    qcT = qcT_f[hr : hr + DH, :]
                    nc.vector.tensor_scalar(
                        out=qcT, in0=qT_h[:, :QL],
                        scalar1=cb_s[hr : hr + DH, hi : hi + 1], scalar2=0.125,
                        op0=mybir.AluOpType.add, op1=mybir.AluOpType.mult,
                    )
                    qpT_f = work.tile([P, QL], BF16, name="qpT_f", tag="qpT_f")
                    qpT = qpT_f[hr : hr + DH, :]
                    nc.vector.tensor_scalar(
                        out=qpT, in0=qT_h[:, :QL],
                        scalar1=pb_s[hr : hr + DH, hi : hi + 1], scalar2=0.125,
                        op0=mybir.AluOpType.add, op1=mybir.AluOpType.mult,
                    )

                    # --- BD_raw -> y_h (row layout, padded rows, col0 = 0) ---
                    y1 = ydram.tile(
                        [(YROWS + 1) * YW], BF16, name=f"y{h}", tag=f"y{h}"
                    )
                    y2d = y1.rearrange("(a b) -> a b", b=YW)
                    wr_insts = []
                    for t in range(2):
                        bw = work.tile([P, YW], BF16, name="bw", tag="bw")
                        nc.gpsimd.memset(bw[:, 0:1], 0.0)
                        for o in range(0, KL, 512):
                            psb = ps_sc.tile([P, 512], F32, name="psb", tag="sc")
                            nc.tensor.matmul(
                                psb, qpT[:, t * P : (t + 1) * P],
                                rT_h[:, o : o + 512], start=True, stop=True,
                            )
                            nc.scalar.copy(out=bw[:, 1 + o : 513 + o], in_=psb)
                        wr_insts.append(
                            nc.gpsimd.dma_start(
                                out=y2d[t * P : (t + 1) * P, :], in_=bw
                            )
                        )
                    wr_insts.append(
                        nc.gpsimd.dma_start(
                            out=y2d[QL : QL + 1, :], in_=halo_s[0:1, h, :]
                        )
                    )

                    # --- transposed BD read via one DMA xbar transpose:
                    # in [256a, 1024kb] -> out [128p, 8c, 256a] (3D out folds the
                    # extra dim into logical partitions). dma_start_transpose
                    # lowers its APs eagerly, so Tile cannot track the y1/bdT
                    # accesses -- add the write->read->use deps explicitly.
                    bdT = work.tile([P, 8, QL], BF16, name="bdT", tag="bdT")
                    tr_inst = nc.sync.dma_start(
                        out=bdT,
                        in_=y1[bass.ds(sb_sv, QL * KL)].rearrange(
                            "(a b) -> a b", b=KL
                        ),
                        transpose=True,
                    )
                    for wi in wr_insts:
                        add_dep_helper(tr_inst.ins, wi.ins, reason="bdT read waits y writes")
                    if len(ad_hist) >= 3:
                        # bdT tiles rotate through 3 slots; the untracked
                        # transpose write must wait for slot h-3's readers
                        for prev_ad in ad_hist[-3]:
                            add_dep_helper(
                                tr_inst.ins, prev_ad.ins,
                                reason="bdT slot reuse waits prior readers",
                            )
                    cur_ads = []
                    ad_hist.append(cur_ads)

                    # --- S^T = AC^T + BDshift^T, exp, sums via ones-matmul ---
                    pT = work.tile([P, 8, QL], BF16, name="pT", tag="pT")
                    sT = work.tile([P, 8, QL], BF16, name="sT", tag="sT")
                    for cb4 in range(2):
                        psa = ps_sa.tile([P, 4, QL], F32, name="psa", tag="sa")
                        for cc in range(4):
                            c = cb4 * 4 + cc
                            nc.tensor.matmul(
                                psa[:, cc, :], kT_h[:, c * P : (c + 1) * P], qcT,
                                start=True, stop=True,
                            )
                        ad = nc.vector.tensor_add(
                            out=sT[:, cb4 * 4 : cb4 * 4 + 4, :], in0=psa,
                            in1=bdT[:, cb4 * 4 : cb4 * 4 + 4, :],
                        )
                        add_dep_helper(ad.ins, tr_inst.ins, reason="s add waits bdT read")
                        cur_ads.append(ad)
                        if use_mask:
                            nc.vector.tensor_add(
                                out=sT[:, cb4 * 4 : cb4 * 4 + 4, :],
                                in0=sT[:, cb4 * 4 : cb4 * 4 + 4, :],
                                in1=mask_s[:, cb4 * 4 : cb4 * 4 + 4, :],
                            )
                    pssum = ps_ms.tile([P, QL], F32, name="pssum", tag="msum")
                    for half in range(2):
                        nc.scalar.activation(
                            out=pT[:, half * 4 : half * 4 + 4, :],
                            in_=sT[:, half * 4 : half * 4 + 4, :],
                            func=mybir.ActivationFunctionType.Exp,
                        )
                        for cc in range(4):
                            c = half * 4 + cc
                            nc.tensor.matmul(
                                pssum, ones_m, pT[:, c, :],
                                start=(c == 0), stop=(c == 7),
                            )
                    rcpb = work.tile([P, QL], F32, name="rcpb", tag="rcpb")
                    nc.vector.reciprocal(out=rcpb, in_=pssum)

                    # --- ctxT_h = (v_h^T @ pT) * rcpb ---
                    psc_f = ps_ms.tile([P, QL], F32, name="psc_f", tag="cx")
                    psc = psc_f[hr : hr + DH, :]
                    for j in range(8):
                        nc.tensor.matmul(
                            psc, v_s[:, j, h * DH : (h + 1) * DH], pT[:, j, :],
                            start=(j == 0), stop=(j == 7),
                        )
                    nc.vector.tensor_mul(
                        out=ctxT_s[hr : hr + DH, hi, :], in0=psc,
                        in1=rcpb[hr : hr + DH, :],
                    )

            abuf_ctx.__exit__(None, None, None)

            # =================== PHASE C: Wo + LN1 ===================
            with (
                tc.tile_pool(name="pc_w", bufs=3) as pc_w,
                tc.tile_pool(name="pc_work", bufs=2) as cwork,
                tc.tile_pool(name="pc_ps", bufs=2, space="PSUM") as pc_ps,
                tc.tile_pool(name="pc_ps_tp", bufs=2, space="PSUM") as pc_ps_tp,
            ):

                def layer_norm(dst, u, gname, bname):
                    # dst, u: [P, D] f32 sbuf aps
                    stats = cwork.tile([P, 2, 6], F32, name="stats", tag="stats")
                    for sg in range(2):
                        nc.vector.bn_stats(
                            out=stats[:, sg, :], in_=u[:, sg * 512 : (sg + 1) * 512]
                        )
                    mv = cwork.tile([P, 2], F32, name="mv", tag="mv")
                    nc.vector.bn_aggr(out=mv, in_=stats)
                    rstd = cwork.tile([P, 1], F32, name="rstd", tag="rstd")
                    nc.scalar.activation(
                        out=rstd, in_=mv[:, 1:2],
                        func=mybir.ActivationFunctionType.Sqrt, bias=eps_t,
                    )
                    nc.vector.reciprocal(out=rstd, in_=rstd)
                    nc.vector.tensor_scalar(
                        out=dst, in0=u, scalar1=mv[:, 0:1], scalar2=rstd,
                        op0=mybir.AluOpType.subtract, op1=mybir.AluOpType.mult,
                    )
                    if gname:
                        nc.vector.tensor_mul(out=dst, in0=dst, in1=rv[gname])
                    if bname:
                        nc.vector.tensor_add(out=dst, in0=dst, in1=rv[bname])

                for t in range(2):
                    pso = pc_ps.tile([P, D], F32, name="pso", tag="pso")
                    for j in range(8):
                        mm_acc(nc, pso, ctxT_s[:, j, t * P : (t + 1) * P],
                               wo_s[:, j, :], first=(j == 0), last=(j == 7))
                    u1 = cwork.tile([P, D], F32, name="u1", tag="u1")
                    nc.vector.tensor_add(out=u1, in0=pso, in1=xres_s[:, t, :])
                    layer_norm(
                        y_s[:, t, :], u1,
                        "g1r" if use_g1 else None, "be1r" if use_be1 else None,
                    )
                    if use_npm:
                        nc.vector.tensor_scalar_mul(
                            out=y_s[:, t, :], in0=y_s[:, t, :],
                            scalar1=npm_s[:, t : t + 1],
                        )
                    ybf = cwork.tile([P, D], BF16, name="ybf", tag="ybf")
                    nc.scalar.copy(out=ybf, in_=y_s[:, t, :])
                    for j in range(8):
                        tp = pc_ps_tp.tile([P, P], BF16, name="tp2", tag="tp2")
                        nc.tensor.transpose(tp, ybf[:, j * P : (j + 1) * P], ident)
                        nc.scalar.copy(out=yT_s[:, j, t * P : (t + 1) * P], in_=tp)

                # =================== PHASE D: MLP + LN2 ===================
                with (
                    tc.tile_pool(name="pd_w", bufs=3) as pd_w,
                    tc.tile_pool(name="pd_h", bufs=1) as pd_h,
                    tc.tile_pool(name="pd_ps_h", bufs=2, space="PSUM") as pd_ps_h,
                ):
                    h1T = pd_h.tile([P, F // P, QL], BF16)
                    for jb in range(8):  # batches of 4 f-chunks
                        w1 = pd_w.tile([P, 8, 512], BF16, name="w1", tag="w1")
                        (nc.scalar, nc.gpsimd)[jb % 2].dma_start(
                            out=w1,
                            in_=W1.rearrange("(c p) n -> p c n", p=P)[
                                :, :, jb * 512 : (jb + 1) * 512
                            ],
                        )
                        for jj in range(4):
                            j = jb * 4 + jj
                            psh1 = pd_ps_h.tile([P, QL], F32, name="psh1", tag="psh1")
                            for kc in range(8):
                                nc.tensor.matmul(
                                    psh1, w1[:, kc, jj * P : (jj + 1) * P],
                                    yT_s[:, kc, :],
                                    start=(kc == 0), stop=(kc == 7),
                                )
                            nc.scalar.activation(
                                out=h1T[:, j, :], in_=psh1,
                                func=mybir.ActivationFunctionType.Relu,
                                bias=b1_s[:, j : j + 1],
                            )
                    ps2_t = [
                        pc_ps.tile([P, D], F32, name=f"ps2_{t}", tag="pso")
                        for t in range(2)
                    ]
                    for jb in range(8):
                        w2 = pd_w.tile([P, 4, D], BF16, name="w2", tag="w2")
                        (nc.gpsimd, nc.scalar)[jb % 2].dma_start(
                            out=w2,
                            in_=W2.rearrange("(c p) n -> p c n", p=P)[
                                :, jb * 4 : (jb + 1) * 4, :
                            ],
                        )
                        for jj in range(4):
                            j = jb * 4 + jj
                            for t in range(2):
                                mm_acc(nc, ps2_t[t], h1T[:, j, t * P : (t + 1) * P],
                                       w2[:, jj, :],
                                       first=(j == 0), last=(j == F // P - 1))
                    for t in range(2):
                        ps2 = ps2_t[t]
                        u2 = cwork.tile([P, D], F32, name="u2", tag="u1")
                        nc.vector.tensor_add(out=u2, in0=ps2, in1=y_s[:, t, :])
                        if use_b2:
                            nc.vector.tensor_add(out=u2, in0=u2, in1=rv["b2r"])
                        o2 = cwork.tile([P, D], F32, name="o2", tag="o2")
                        layer_norm(
                            o2, u2,
                            "g2r" if use_g2 else None, "be2r" if use_be2 else None,
                        )
                        if use_npm:
                            nc.vector.tensor_scalar_mul(
                                out=o2, in0=o2, scalar1=npm_s[:, t : t + 1]
                            )
                        nc.gpsimd.dma_start(
                            out=out_t.rearrange("(t p) d -> p t d", p=P)[:, t, :],
                            in_=o2,
                        )

    nc.compile()
    return nc


def _host_prep(inputs):
    """Shared (core-independent) host prep: bf16 casts + pe table."""
    f32 = lambda x: np.asarray(x, np.float32)
    bf = lambda x: np.asarray(x, np.float32).astype(NP_BF16)

    pos = np.arange(KL - 1, -1, -1, dtype=np.float32)
    inv = (1.0 / (10000.0 ** (np.arange(0, D, 2, dtype=np.float32) / D))).astype(
        np.float32
    )
    ang = pos[:, None] * inv
    pe = np.concatenate([np.sin(ang), np.cos(ang)], axis=-1).astype(np.float32)

    shared = {
        "peT": bf(pe.T),
        "Wq": bf(inputs["Wq"]), "Wk": bf(inputs["Wk"]),
        "Wv": bf(inputs["Wv"]), "Wr": bf(inputs["Wr"]),
        "Wo": bf(inputs["Wo"]), "W1": bf(inputs["W1"]), "W2": bf(inputs["W2"]),
        "cbt": f32(inputs["content_bias"]).reshape(HK).reshape(8, P).T.copy(),
        "pbt": f32(inputs["position_bias"]).reshape(HK).reshape(8, P).T.copy(),
        "b1t": f32(inputs["b1"]).reshape(F // P, P).T.copy(),
    }
    return shared


def _run(inputs, trace=False):
    x = np.asarray(inputs["layer_input"], np.float32)
    mem = np.asarray(inputs["memory"], np.float32)
    npm = np.asarray(inputs["non_pad_mask"], np.float32)
    mask = np.asarray(inputs["slf_attn_mask"])
    g1 = np.asarray(inputs["ln1_g"], np.float32)
    be1 = np.asarray(inputs["ln1_b"], np.float32)
    g2 = np.asarray(inputs["ln2_g"], np.float32)
    be2 = np.asarray(inputs["ln2_b"], np.float32)
    b2 = np.asarray(inputs["b2"], np.float32)

    flags = (
        bool(mask.any()),
        not bool(np.all(npm == 1.0)),
        not bool(np.all(g1 == 1.0)),
        bool(be1.any()),
        not bool(np.all(g2 == 1.0)),
        bool(be2.any()),
        bool(b2.any()),
    )
    if flags not in _cache:
        _cache[flags] = build(flags)
    nc = _cache[flags]
    use_mask, use_npm, use_g1, use_be1, use_g2, use_be2, use_b2 = flags

    shared = _host_prep(inputs)
    in_maps = []
    for c in range(NCORES):
        b, qh = c // 2, c % 2
        lo = QL * qh
        kv = np.concatenate([mem[b], x[b]], axis=0)  # [KL, D]
        xq = np.zeros((YROWS, D), np.float32)
        hi = min(lo + YROWS, Q)
        xq[: hi - lo] = x[b, lo:hi]
        m = dict(shared)
        m["sb"] = np.array([[512 - lo]], np.uint32)
        m["xqT"] = xq.T.astype(NP_BF16)
        m["kvT"] = kv.T.astype(NP_BF16)
        m["xres"] = np.ascontiguousarray(x[b, lo : lo + QL])
        if use_mask:
            m["maskadd"] = np.where(
                mask[b, lo : lo + QL], np.float32(-1e9), np.float32(0)
            ).astype(np.float32)
        if use_npm:
            m["npmt"] = npm[b, lo : lo + QL, 0].reshape(2, P).T.copy()
        if use_g1:
            m["g1r"] = g1.reshape(1, D).copy()
        if use_be1:
            m["be1r"] = be1.reshape(1, D).copy()
        if use_g2:
            m["g2r"] = g2.reshape(1, D).copy()
        if use_be2:
            m["be2r"] = be2.reshape(1, D).copy()
        if use_b2:
            m["b2r"] = b2.reshape(1, D).copy()
        in_maps.append(m)

    res = run_bass_kernel_spmd(nc, in_maps, core_ids=list(range(NCORES)), trace=trace)
    out = np.empty((B, Q, D), np.float32)
    for c in range(NCORES):
        b, qh = c // 2, c % 2
        out[b, QL * qh : QL * (qh + 1)] = res.results[c]["out"]
    return out, res


def kernel(**inputs):
    out, _ = _run(inputs, trace=False)
    return out
